# revision 13
# baseline (speedup 1.0000x reference)
"""DyRep classifier Bass kernel for 8 Trainium2 NeuronCores.

Strategy (self-contained; shapes hardcoded for the target problem):
  - The output depends only on per-label-node rows of (memory_buf,
    node_state[post-event], last_seen[post-event], node_features).
  - For nodes NOT touched by the event batch, the decayed blend
    mem + state*exp(-decay*(T-last_seen)) depends only on per-node data,
    so the host folds it into one bf16 "combined" vector per node while
    packing the table. Features are packed as fp8(e3m4, x32 scale).
    Untouched row = [combined bf16 x128 | feat fp8 x172 | pad] = 512 B
    (the DMA-descriptor efficiency floor: the transpose-gather writes
    SBUF in 256B blocks, cost scales with blocks per row).
  - Touched nodes (hit by the event batch) need the GRU applied on
    device: compact per-chunk table of label-touched rows,
    [mem x128 | state x128 | feat fp8 | pad] = 768 B.
  - Tables sharded row-wise into 16 chunks of 31250 rows
    (int16-indexable); each of the 8 cores owns 2 chunks. Host routes
    each unique label node to its owner (core, chunk) and splits
    touched/untouched; device gathers+computes; host unpermutes.
  - Device per gather: dma_gather(transpose=True) delivers rows
    feature-major; then W1@combined + (W1@W_feat)@feat via 3 matmuls
    (fp8 feature halves addressed with stride-2 column APs), ACT
    relu+bias, W2 classifier, bf16 output.
"""

import functools
import numpy as np
import ml_dtypes

import concourse.bass as bass
import concourse.mybir as mybir
import concourse.tile as tile
from concourse import bacc
from concourse.bass_utils import run_bass_kernel_spmd

BF16 = ml_dtypes.bfloat16
FP8 = ml_dtypes.float8_e4m3
F8SCALE = 32.0
BIG = F8SCALE * F8SCALE   # PSUM scale: W1 pass x1024, undone in the relu ACT

# Problem dims (fixed by the task)
N = 500000
H = 128
F = 172
C = 50
B = 200000

NCORES = 8
NCHUNK = 16                  # index chunks (int16 addressing limit)
CH = N // NCHUNK             # 31250 rows per chunk
ROW_U = 256                  # bf16 units per untouched row (512 B)
ROW_T = 384                  # bf16 units per touched row (768 B)
# transpose dma_gather num_idxs is HW-capped at ~1016 (896 largest %128).
# The gather's cost is per-INDEX (Q7 descriptor-gen ucode, ~10ns/idx/queue),
# independent of row bytes; 4 SWDGE queues run gathers concurrently.
G_U = 896                    # untouched occurrences per gather (512B rows)
G_T = 896                    # touched occurrences per gather (768B rows)
SMAX = 512                   # max compute-slice cols (PSUM bank = 512 f32)
NQ = 4                       # SWDGE queues (ucode max)

f32 = mybir.dt.float32
bf16 = mybir.dt.bfloat16
fp8e4 = mybir.dt.float8e4
i16 = mybir.dt.int16
AF = mybir.ActivationFunctionType
OP = mybir.AluOpType


def _wrap_idxs(idx: np.ndarray) -> np.ndarray:
    """Wrap a flat int16 index list into the [128, n/16] SWDGE layout:
    element j at [j%16, j//16], replicated into all 8 16-partition groups."""
    n = idx.shape[0]
    assert n % 16 == 0
    cols = n // 16
    t = np.empty((128, cols), dtype=np.int16)
    blk = idx.reshape(cols, 16).T  # [16, cols]
    for k in range(8):
        t[k * 16:(k + 1) * 16, :] = blk
    return t


def _sched(total: int, gmax: int) -> list[int]:
    out = []
    rem = total
    while rem > 0:
        g = min(gmax, rem)
        out.append(g)
        rem -= g
    return out


def build_program(u_pad: int, t_pad: int, tt_rows: int):
    """Build + compile the SPMD Bass program. Cached by padded sizes."""
    nc = bacc.Bacc("TRN2", target_bir_lowering=False, debug=False,
                   num_devices=NCORES, num_swdge_queues=NQ)

    dt_in = {}

    def din(name, shape, dt):
        dt_in[name] = nc.dram_tensor(name, shape, dt, kind="ExternalInput").ap()
        return dt_in[name]

    tab_a = din("tab_a", (CH, ROW_U), bf16)
    tab_b = din("tab_b", (CH, ROW_U), bf16)
    tabt_a = din("tabt_a", (tt_rows, ROW_T), bf16)
    tabt_b = din("tabt_b", (tt_rows, ROW_T), bf16)
    uidx_a = din("uidx_a", (128, u_pad // 16), i16)
    uidx_b = din("uidx_b", (128, u_pad // 16), i16)
    tidx_a = din("tidx_a", (128, t_pad // 16), i16)
    tidx_b = din("tidx_b", (128, t_pad // 16), i16)

    w1t = din("w1t", (128, 128), bf16)       # W1.T * BIG
    wfdr = din("wfdr", (128, 2, 128), fp8e4)  # DoubleRow (W1@W_feat).T * F8SCALE
    w2t = din("w2t", (128, 64), bf16)        # W2.T zero-padded to 64 rows
    whhrt = din("whhrt", (128, 128), bf16)   # W_hh[0:128].T
    whhzt = din("whhzt", (128, 128), bf16)   # W_hh[128:256].T
    whhnt = din("whhnt", (128, 128), bf16)   # W_hh[256:384].T
    b1p = din("b1p", (128, 1), f32)          # b1 + W1 @ b_feat
    b2v = din("b2v", (C, 1), f32)
    b2d = din("b2d", (128, 1), f32)          # b2 at rows 0:C and 64:64+C
    c_r = din("c_r", (128, 1), f32)          # gi_r + b_hh_r
    c_z = din("c_z", (128, 1), f32)          # gi_z + b_hh_z
    gin = din("gin", (128, 1), f32)          # gi_n
    bhn = din("bhn", (128, 1), f32)          # b_hh_n
    dect = din("dect", (128, 1), f32)        # exp(-relu(decay)*(T - t0))
    sbig = din("sbig", (128, 1), f32)        # 1/BIG

    totcol = 2 * (u_pad + t_pad)
    out = nc.dram_tensor("out", (C, totcol), bf16, kind="ExternalOutput").ap()

    u_gs = _sched(u_pad, G_U)
    t_gs_a = [128] + _sched(t_pad - 128, G_T)  # peel: early ucode lib load
    t_gs_b = _sched(t_pad, G_T)

    class W:
        pass

    with tile.TileContext(nc) as tc:
        with tc.tile_pool(name="wpool", bufs=1) as wp:
            for name in ("tidx_a", "tidx_b", "uidx_a", "uidx_b",
                         "w1t", "wfdr", "w2t", "whhrt", "whhzt",
                         "whhnt", "b1p", "b2v", "b2d", "c_r", "c_z", "gin",
                         "bhn", "dect", "sbig"):
                ap = dt_in[name]
                t = wp.tile(list(ap.shape), ap.dtype, tag=name)
                nc.sync.dma_start(t[:], ap[:])
                setattr(W, name, t)

            def tail(sb, ps2, X, nb, c0, S, rhs_t1, o_sl):
                """Classifier tail (single slice, touched path): p_h1 =
                W1@t1 + Wc@feat; relu; W2; bias -> o_sl (bf16)."""
                X8 = X[:, nb - 1, :].bitcast(fp8e4)  # [128, 2G]
                X83 = X8.rearrange("p (j two) -> p two j", two=2)
                p_h1 = ps2.tile([128, SMAX], f32, tag="h1")
                nc.tensor.matmul(p_h1[:, :S], lhsT=W.w1t[:], rhs=rhs_t1,
                                 start=True, stop=False)
                nc.tensor.matmul(p_h1[:, :S], lhsT=W.wfdr[:],
                                 rhs=X83[:, :, bass.ds(c0, S)],
                                 start=False, stop=True,
                                 perf_mode=mybir.MatmulPerfMode.DoubleRow)
                h1 = sb.tile([128, SMAX], bf16, tag="h1s")
                nc.scalar.activation(h1[:, :S], p_h1[:, :S], AF.Relu,
                                     bias=W.b1p[:], scale=W.sbig[:])
                p_o = ps2.tile([64, SMAX], f32, tag="out")
                nc.tensor.matmul(p_o[:, :S], lhsT=W.w2t[:], rhs=h1[:, :S],
                                 start=True, stop=True)
                nc.vector.tensor_scalar_add(o_sl, p_o[0:C, :S], W.b2v[:, 0:1])

            def upair(sb, ps2, X, c0a, S, Sb, osb):
                """Two equal untouched slices of one gather, weight-batched.
                The two W2 outputs land in one PSUM tile at partition rows
                0:64 and 64:128 (tile_position col packing) so one DVE op
                applies the bias for both."""
                c0b = c0a + S
                X8 = X[:, 1, :].bitcast(fp8e4)
                X83 = X8.rearrange("p (j two) -> p two j", two=2)
                ph_a = ps2.tile([128, SMAX], f32, tag="h1a")
                nc.tensor.matmul(ph_a[:, :S], lhsT=W.w1t[:],
                                 rhs=X[:, 0, bass.ds(c0a, S)],
                                 start=True, stop=False)
                if Sb:
                    ph_b = ps2.tile([128, SMAX], f32, tag="h1b")
                    nc.tensor.matmul(ph_b[:, :S], lhsT=W.w1t[:],
                                     rhs=X[:, 0, bass.ds(c0b, S)],
                                     start=True, stop=False)
                nc.tensor.matmul(ph_a[:, :S], lhsT=W.wfdr[:],
                                 rhs=X83[:, :, bass.ds(c0a, S)],
                                 start=False, stop=True,
                                 perf_mode=mybir.MatmulPerfMode.DoubleRow)
                if Sb:
                    nc.tensor.matmul(ph_b[:, :S], lhsT=W.wfdr[:],
                                     rhs=X83[:, :, bass.ds(c0b, S)],
                                     start=False, stop=True,
                                     perf_mode=mybir.MatmulPerfMode.DoubleRow)
                h1a = sb.tile([128, SMAX], bf16, tag="h1sa")
                nc.scalar.activation(h1a[:, :S], ph_a[:, :S], AF.Relu,
                                     bias=W.b1p[:], scale=W.sbig[:])
                if Sb:
                    h1b = sb.tile([128, SMAX], bf16, tag="h1sb")
                    nc.scalar.activation(h1b[:, :S], ph_b[:, :S], AF.Relu,
                                         bias=W.b1p[:], scale=W.sbig[:])
                p_o = ps2.tile([128, SMAX], f32, tag="out2")
                nc.tensor.matmul(p_o[0:64, :S], lhsT=W.w2t[:], rhs=h1a[:, :S],
                                 start=True, stop=True, tile_position=(0, 0))
                if Sb:
                    nc.tensor.matmul(p_o[64:128, :S], lhsT=W.w2t[:],
                                     rhs=h1b[:, :S],
                                     start=True, stop=True,
                                     tile_position=(0, 64))
                    nc.vector.tensor_scalar_add(
                        osb[:, :S], p_o[:, :S], W.b2d[:, 0:1])
                else:
                    nc.vector.tensor_scalar_add(
                        osb[0:C, :S], p_o[0:C, :S], W.b2v[:, 0:1])

            def slice_t(sb, ps, ps2, X, c0, S, o_sl):
                sl = bass.ds(c0, S)
                stT = X[:, 1, sl]
                p_r = ps.tile([128, SMAX], f32, tag="gr")
                nc.tensor.matmul(p_r[:, :S], lhsT=W.whhrt[:], rhs=stT,
                                 start=True, stop=True)
                p_z = ps.tile([128, SMAX], f32, tag="gz")
                nc.tensor.matmul(p_z[:, :S], lhsT=W.whhzt[:], rhs=stT,
                                 start=True, stop=True)
                p_n = ps.tile([128, SMAX], f32, tag="gn")
                nc.tensor.matmul(p_n[:, :S], lhsT=W.whhnt[:], rhs=stT,
                                 start=True, stop=True)
                r = sb.tile([128, SMAX], f32, tag="r")
                nc.scalar.activation(r[:, :S], p_r[:, :S], AF.Sigmoid,
                                     bias=W.c_r[:])
                z = sb.tile([128, SMAX], f32, tag="z")
                nc.scalar.activation(z[:, :S], p_z[:, :S], AF.Sigmoid,
                                     bias=W.c_z[:])
                hn = sb.tile([128, SMAX], f32, tag="hn")
                nc.scalar.activation(hn[:, :S], p_n[:, :S], AF.Identity,
                                     bias=W.bhn[:])
                rn = sb.tile([128, SMAX], f32, tag="rn")
                nc.vector.tensor_tensor(out=rn[:, :S], in0=r[:, :S],
                                        in1=hn[:, :S], op=OP.mult)
                n = sb.tile([128, SMAX], f32, tag="n")
                nc.scalar.activation(n[:, :S], rn[:, :S], AF.Tanh,
                                     bias=W.gin[:])
                d = sb.tile([128, SMAX], f32, tag="d")
                nc.vector.tensor_tensor(out=d[:, :S], in0=stT, in1=n[:, :S],
                                        op=OP.subtract)
                zd = sb.tile([128, SMAX], f32, tag="zd")
                nc.vector.tensor_tensor(out=zd[:, :S], in0=z[:, :S],
                                        in1=d[:, :S], op=OP.mult)
                ns = sb.tile([128, SMAX], f32, tag="ns")
                nc.vector.tensor_tensor(out=ns[:, :S], in0=n[:, :S],
                                        in1=zd[:, :S], op=OP.add)
                sstate = sb.tile([128, SMAX], bf16, tag="sstate")
                nc.vector.tensor_scalar_mul(sstate[:, :S], ns[:, :S],
                                            W.dect[:, 0:1])
                t1 = sb.tile([128, SMAX], bf16, tag="t1")
                nc.vector.tensor_tensor(out=t1[:, :S], in0=sstate[:, :S],
                                        in1=X[:, 0, sl], op=OP.add)
                tail(sb, ps2, X, 3, c0, S, t1[:, :S], o_sl)

            qctr = [0]

            def gather(gp, table_ap, idx_tile, goff, g, nb, tag, bufs):
                q = qctr[0] % NQ
                qctr[0] += 1
                X = gp.tile([128, nb, g], bf16, tag=f"{tag}{q}_{g}",
                            bufs=bufs)
                nc.gpsimd.dma_gather(
                    out_ap=X[:],
                    in_ap=table_ap[:],
                    idxs_ap=idx_tile[:, bass.ds(goff // 16, g // 16)],
                    num_idxs=g,
                    num_idxs_reg=g,
                    elem_size=nb * 128,
                    transpose=True,
                    queue_num=q,
                )
                return X

            def tstream(gp, sb, ps, ps2, table_ap, idx_tile, gsizes, col0):
                goff = 0
                for g in gsizes:
                    X = gather(gp, table_ap, idx_tile, goff, g, 3, "gt", 2)
                    osb = gp.tile([C, G_T], bf16, tag="osbt", bufs=2)
                    c0 = 0
                    for S in _sched(g, SMAX):
                        slice_t(sb, ps, ps2, X, c0, S,
                                osb[:, bass.ds(c0, S)])
                        c0 += S
                    nc.sync.dma_start(
                        out[:, bass.ds(col0 + goff, g)], osb[:, :g])
                    goff += g

            def ustream(gp, sb, ps2, table_ap, idx_tile, col0):
                goff = 0
                for g in u_gs:
                    X = gather(gp, table_ap, idx_tile, goff, g, 2, "gu",
                               4 if g == G_U else 2)
                    if g > SMAX:
                        Sa = Sb = g // 2
                    else:
                        Sa, Sb = g, 0
                    osb = gp.tile([128, SMAX], bf16, tag="osbu", bufs=6)
                    upair(sb, ps2, X, 0, Sa, Sb, osb)
                    nc.sync.dma_start(
                        out[:, bass.ds(col0 + goff, Sa)], osb[0:C, :Sa])
                    if Sb:
                        nc.sync.dma_start(
                            out[:, bass.ds(col0 + goff + Sa, Sb)],
                            osb[64:64 + C, :Sb])
                    goff += g

            # PE warm-up burst: dummy matmuls against the already-loaded
            # uidx bytes (bitcast bf16) keep the PE HAM active through the
            # gather startup so real matmuls run un-throttled (2.4 GHz).
            with tc.tile_pool(name="wmps", bufs=1, space="PSUM") as wps:
                wrhs = W.uidx_a[:].bitcast(bf16)
                wcols = min(SMAX, u_pad // 16)
                pwarm = wps.tile([128, SMAX], f32, tag="warm")
                for _ in range(16):
                    nc.tensor.matmul(pwarm[:, :wcols], lhsT=W.w1t[:],
                                     rhs=wrhs[:, :wcols],
                                     start=True, stop=True)

            # Touched first: its serial GRU chain drains while untouched
            # gathers already issue from the shared gather pool.
            with tc.tile_pool(name="gp", bufs=4) as gp:
                with tc.tile_pool(name="sbt", bufs=2) as sb, \
                     tc.tile_pool(name="pst", bufs=1, space="PSUM") as ps, \
                     tc.tile_pool(name="pst2", bufs=2, space="PSUM") as ps2:
                    tstream(gp, sb, ps, ps2, tabt_a, W.tidx_a, t_gs_a, 0)
                    tstream(gp, sb, ps, ps2, tabt_b, W.tidx_b, t_gs_b, t_pad)
                with tc.tile_pool(name="sbu", bufs=3) as sb, \
                     tc.tile_pool(name="psu2", bufs=2, space="PSUM") as ps2:
                    ustream(gp, sb, ps2, tab_a, W.uidx_a, 2 * t_pad)
                    ustream(gp, sb, ps2, tab_b, W.uidx_b, 2 * t_pad + u_pad)

    nc.compile()
    return nc


@functools.lru_cache(maxsize=4)
def _cached_program(u_pad, t_pad, tt_rows):
    return build_program(u_pad, t_pad, tt_rows)


def _round_up(x, m):
    return ((x + m - 1) // m) * m


def _pack_feat_u16(feats: np.ndarray) -> np.ndarray:
    """fp8(e3m4, x32) encode features and pack byte pairs into uint16
    (little-endian: feature 2p low byte, 2p+1 high byte -> unit p)."""
    f8 = (feats.astype(np.float32) * F8SCALE).astype(FP8).view(np.uint8)
    return f8[:, 0::2].astype(np.uint16) | (
        f8[:, 1::2].astype(np.uint16) << 8)


def _prepare(label_nodes, src, dst, t, msg, memory_buf, node_state, last_seen,
             node_features, decay, W_msg, b_msg, W_ih, W_hh, b_ih, b_hh,
             W_feat, b_feat, W1, b1, W2, b2, current_time):
    """Host-side routing/packing. Returns (in_maps, meta)."""
    label_nodes = np.asarray(label_nodes)

    # ---- host: event-level scalars (O(1) work) ----
    t0 = float(np.asarray(t)[0])
    T = float(current_time)
    rdecay = max(float(decay), 0.0)
    event_msg = msg[0].astype(np.float64) @ W_msg.T.astype(np.float64) + b_msg
    gi = event_msg @ W_ih.T.astype(np.float64) + b_ih  # [3H], includes b_ih
    gi = gi.astype(np.float32)
    dec_t = np.float32(np.exp(-rdecay * (T - t0)))

    # ---- host: routing (dedup to unique label nodes) ----
    touched_nodes = np.unique(np.concatenate([src, dst]))
    uniq_vals, inv = np.unique(label_nodes, return_inverse=True)
    is_t = np.isin(uniq_vals, touched_nodes)
    chunk_id = uniq_vals // CH            # 0..15
    local = (uniq_vals % CH).astype(np.int16)

    key = chunk_id.astype(np.int64) * 2 + is_t
    order = np.argsort(key, kind="stable")
    counts = np.bincount(key, minlength=NCHUNK * 2)
    u_counts = counts[0::2]
    t_counts = counts[1::2]
    u_pad = max(_round_up(int(u_counts.max()), 128), 128)
    t_pad = max(_round_up(int(t_counts.max()), 128), 128)

    starts = np.zeros(NCHUNK * 2 + 1, dtype=np.int64)
    np.cumsum(counts, out=starts[1:])
    group_uids = {}  # (chunk, touched) -> unique-label ids in device order
    for ci in range(NCHUNK):
        for tf in (0, 1):
            k = ci * 2 + tf
            group_uids[(ci, tf)] = order[starts[k]:starts[k + 1]]

    # ---- host: untouched packed table [N, 512B] ----
    # combined = mem + state * exp(-decay*(T - last_seen)) (valid when the
    # event did not touch the node; touched labels never read this table)
    dec_all = np.exp(-rdecay * (T - last_seen.astype(np.float32)))
    comb = (memory_buf.astype(np.float32)
            + node_state.astype(np.float32) * dec_all[:, None]).astype(BF16)
    tab = np.zeros((N, ROW_U), dtype=np.uint16)
    tab[:, 0:128] = comb.view(np.uint16)
    tab[:, 128:214] = _pack_feat_u16(node_features)
    tab = tab.view(BF16)

    # ---- host: touched compact tables (label-touched rows per chunk, in
    # device order -> gather indices are just arange) ----
    tt_rows = 16
    t_rows = {}
    for ci in range(NCHUNK):
        nodes = uniq_vals[group_uids[(ci, 1)]]
        t_rows[ci] = nodes
        tt_rows = max(tt_rows, _round_up(nodes.shape[0], 16))

    def build_tabt(nodes):
        tt = np.zeros((tt_rows, ROW_T), dtype=np.uint16)
        nr = nodes.shape[0]
        if nr:
            tt[:nr, 0:128] = memory_buf[nodes].astype(BF16).view(np.uint16)
            tt[:nr, 128:256] = node_state[nodes].astype(BF16).view(np.uint16)
            tt[:nr, 256:342] = _pack_feat_u16(node_features[nodes])
        return tt.view(BF16)

    # ---- host: weights / aux ----
    def bfc(x):
        return np.ascontiguousarray(x, dtype=BF16)

    def f32c(x):
        return np.ascontiguousarray(x, dtype=np.float32).reshape(-1, 1)

    WcT = (W1 @ W_feat).T  # [F, H] — W_feat folded through W1
    wfdr = np.zeros((128, 2, 128), dtype=FP8)
    wfdr.reshape(256, 128)[0:F] = (WcT * F8SCALE).astype(np.float32).astype(FP8)
    aux = {
        "w1t": bfc(W1.T * BIG),
        "wfdr": wfdr,
        "w2t": bfc(np.concatenate(
            [W2.T, np.zeros((128, 64 - C), np.float32)], axis=1)),
        "whhrt": bfc(W_hh[0:128].T),
        "whhzt": bfc(W_hh[128:256].T),
        "whhnt": bfc(W_hh[256:384].T),
        "b1p": f32c(b1 + W1 @ b_feat),
        "b2v": f32c(b2),
        "b2d": f32c(np.concatenate(
            [b2, np.zeros(14, np.float32), b2, np.zeros(14, np.float32)])),
        "c_r": f32c(gi[0:128] + b_hh[0:128]),
        "c_z": f32c(gi[128:256] + b_hh[128:256]),
        "gin": f32c(gi[256:384]),
        "bhn": f32c(b_hh[256:384]),
        "dect": np.full((128, 1), dec_t, dtype=np.float32),
        "sbig": np.full((128, 1), 1.0 / BIG, dtype=np.float32),
    }

    def uidx_input(ci):
        uids = group_uids[(ci, 0)]
        li = np.zeros(u_pad, dtype=np.int16)
        li[:uids.shape[0]] = local[uids]
        return _wrap_idxs(li)

    def tidx_input(ci):
        li = np.zeros(t_pad, dtype=np.int16)
        nr = group_uids[(ci, 1)].shape[0]
        li[:nr] = np.arange(nr, dtype=np.int16)
        return _wrap_idxs(li)

    in_maps = []
    for core in range(NCORES):
        ca, cb = 2 * core, 2 * core + 1
        im = dict(aux)
        im["tab_a"] = tab[ca * CH:(ca + 1) * CH]
        im["tab_b"] = tab[cb * CH:(cb + 1) * CH]
        im["tabt_a"] = build_tabt(t_rows[ca])
        im["tabt_b"] = build_tabt(t_rows[cb])
        im["uidx_a"] = uidx_input(ca)
        im["uidx_b"] = uidx_input(cb)
        im["tidx_a"] = tidx_input(ca)
        im["tidx_b"] = tidx_input(cb)
        in_maps.append(im)

    # column (within a core's output) of each unique label node
    totcol = 2 * (u_pad + t_pad)
    col_of_uniq = np.empty(uniq_vals.shape[0], dtype=np.int64)
    for ci in range(NCHUNK):
        core = ci // 2
        for tf in (0, 1):
            uids = group_uids[(ci, tf)]
            if tf == 1:
                c0 = 0 if (ci % 2) == 0 else t_pad
            else:
                c0 = 2 * t_pad if (ci % 2) == 0 else 2 * t_pad + u_pad
            col_of_uniq[uids] = core * totcol + c0 + np.arange(uids.shape[0])

    meta = {"u_pad": u_pad, "t_pad": t_pad, "tt_rows": tt_rows,
            "col_of_uniq": col_of_uniq, "inv": inv, "nb": label_nodes.shape[0]}
    return in_maps, meta


def _finish(core_outs, meta):
    """Map per-core [C, totcol] bf16 outputs back to label order (f32)."""
    combined = np.concatenate(core_outs, axis=1)  # [C, NCORES*totcol]
    sel = combined[:, meta["col_of_uniq"][meta["inv"]]].T
    return np.ascontiguousarray(sel, dtype=np.float32)


def kernel(**inputs):
    inputs = {k: np.asarray(v) for k, v in inputs.items()}
    in_maps, meta = _prepare(**inputs)
    nc = _cached_program(meta["u_pad"], meta["t_pad"], meta["tt_rows"])
    res = run_bass_kernel_spmd(nc, in_maps, core_ids=list(range(NCORES)))
    return _finish([r["out"] for r in res.results], meta)


# revision 17
# speedup vs baseline: 1.1104x; 1.1104x over previous
"""DyRep classifier Bass kernel for 8 Trainium2 NeuronCores.

Strategy (self-contained; shapes hardcoded for the target problem):
  - The output depends only on per-label-node rows of (memory_buf,
    node_state[post-event], last_seen[post-event], node_features).
  - For nodes NOT touched by the event batch, the decayed blend
    mem + state*exp(-decay*(T-last_seen)) depends only on per-node data,
    so the host folds it into one bf16 "combined" vector per node while
    packing the table. Features are packed as fp8(e3m4, x32 scale).
    Untouched row = [combined bf16 x128 | feat fp8 x172 | pad] = 512 B
    (the DMA-descriptor efficiency floor: the transpose-gather writes
    SBUF in 256B blocks, cost scales with blocks per row).
  - Touched nodes (hit by the event batch) need the GRU applied on
    device: compact per-chunk table of label-touched rows,
    [mem x128 | state x128 | feat fp8 | pad] = 768 B.
  - Tables sharded row-wise into 16 chunks of 31250 rows
    (int16-indexable); each of the 8 cores owns 2 chunks. Host routes
    each unique label node to its owner (core, chunk) and splits
    touched/untouched; device gathers+computes; host unpermutes.
  - Device per gather: dma_gather(transpose=True) delivers rows
    feature-major; then W1@combined + (W1@W_feat)@feat via 3 matmuls
    (fp8 feature halves addressed with stride-2 column APs), ACT
    relu+bias, W2 classifier, bf16 output.
"""

import functools
import numpy as np
import ml_dtypes

import concourse.bass as bass
import concourse.mybir as mybir
import concourse.tile as tile
from concourse import bacc
from concourse.bass_utils import run_bass_kernel_spmd

BF16 = ml_dtypes.bfloat16
FP8 = ml_dtypes.float8_e4m3
F8SCALE = 32.0
BIG = F8SCALE * F8SCALE   # PSUM scale: W1 pass x1024, undone in the relu ACT

# Problem dims (fixed by the task)
N = 500000
H = 128
F = 172
C = 50
B = 200000

NCORES = 8
NCHUNK = 16                  # index chunks (int16 addressing limit)
CH = N // NCHUNK             # 31250 rows per chunk
ROW_U = 256                  # bf16 units per untouched row (512 B)
ROW_T = 384                  # bf16 units per touched row (768 B)
# transpose dma_gather num_idxs is HW-capped at ~1016 (896 largest %128).
# The gather's cost is per-INDEX (Q7 descriptor-gen ucode, ~10ns/idx/queue),
# independent of row bytes; 4 SWDGE queues run gathers concurrently.
G_U = 896                    # untouched occurrences per gather (512B rows)
G_T = 896                    # touched occurrences per gather (768B rows)
SMAX = 512                   # max compute-slice cols (PSUM bank = 512 f32)
NQ = 4                       # SWDGE queues (ucode max)

f32 = mybir.dt.float32
bf16 = mybir.dt.bfloat16
fp8e4 = mybir.dt.float8e4
i16 = mybir.dt.int16
AF = mybir.ActivationFunctionType
OP = mybir.AluOpType


def _wrap_idxs(idx: np.ndarray) -> np.ndarray:
    """Wrap a flat int16 index list into the [128, n/16] SWDGE layout:
    element j at [j%16, j//16], replicated into all 8 16-partition groups."""
    n = idx.shape[0]
    assert n % 16 == 0
    cols = n // 16
    t = np.empty((128, cols), dtype=np.int16)
    blk = idx.reshape(cols, 16).T  # [16, cols]
    for k in range(8):
        t[k * 16:(k + 1) * 16, :] = blk
    return t


def _sched(total: int, gmax: int) -> list[int]:
    out = []
    rem = total
    while rem > 0:
        g = min(gmax, rem)
        out.append(g)
        rem -= g
    return out


def build_program(u_pad: int, t_pad: int, tt_rows: int):
    """Build + compile the SPMD Bass program. Cached by padded sizes."""
    nc = bacc.Bacc("TRN2", target_bir_lowering=False, debug=False,
                   num_devices=NCORES, num_swdge_queues=NQ)

    dt_in = {}

    def din(name, shape, dt):
        dt_in[name] = nc.dram_tensor(name, shape, dt, kind="ExternalInput").ap()
        return dt_in[name]

    tab_a = din("tab_a", (CH, ROW_U), bf16)
    tab_b = din("tab_b", (CH, ROW_U), bf16)
    tabt_a = din("tabt_a", (tt_rows, ROW_T), bf16)
    tabt_b = din("tabt_b", (tt_rows, ROW_T), bf16)
    uidx_a = din("uidx_a", (128, u_pad // 16), i16)
    uidx_b = din("uidx_b", (128, u_pad // 16), i16)
    tidx_a = din("tidx_a", (128, t_pad // 16), i16)
    tidx_b = din("tidx_b", (128, t_pad // 16), i16)

    w1t = din("w1t", (128, 128), bf16)       # W1.T * BIG
    wfdr = din("wfdr", (128, 2, 128), fp8e4)  # DoubleRow (W1@W_feat).T * F8SCALE
    w2t = din("w2t", (128, 64), bf16)        # W2.T zero-padded to 64 rows
    whhrt = din("whhrt", (128, 128), bf16)   # W_hh[0:128].T
    whhzt = din("whhzt", (128, 128), bf16)   # W_hh[128:256].T
    whhnt = din("whhnt", (128, 128), bf16)   # W_hh[256:384].T
    b1p = din("b1p", (128, 1), f32)          # b1 + W1 @ b_feat
    b2v = din("b2v", (C, 1), f32)
    b2d = din("b2d", (128, 1), f32)          # b2 at rows 0:C and 64:64+C
    c_r = din("c_r", (128, 1), f32)          # gi_r + b_hh_r
    c_z = din("c_z", (128, 1), f32)          # gi_z + b_hh_z
    gin = din("gin", (128, 1), f32)          # gi_n
    bhn = din("bhn", (128, 1), f32)          # b_hh_n
    dect = din("dect", (128, 1), f32)        # exp(-relu(decay)*(T - t0))
    sbig = din("sbig", (128, 1), f32)        # 1/BIG

    totcol = 2 * (u_pad + t_pad)
    out = nc.dram_tensor("out", (C, totcol), bf16, kind="ExternalOutput").ap()

    u_gs = _sched(u_pad, G_U)
    t_gs = _sched(t_pad, G_T)

    class W:
        pass

    with tile.TileContext(nc) as tc:
        with tc.tile_pool(name="wpool", bufs=1) as wp:
            # gather indices + warmup deps first: gathers and the PE
            # warm-up block on these.
            for name in ("tidx_a", "uidx_a", "w1t", "tidx_b", "uidx_b",
                         "whhrt", "whhzt", "whhnt", "c_r", "c_z", "gin",
                         "bhn", "dect", "wfdr", "w2t", "b1p", "b2v", "b2d",
                         "sbig"):
                ap = dt_in[name]
                t = wp.tile(list(ap.shape), ap.dtype, tag=name)
                nc.sync.dma_start(t[:], ap[:])
                setattr(W, name, t)

            def tail(sb, ps2, X, nb, c0, S, rhs_t1, o_sl):
                """Classifier tail (single slice, touched path): p_h1 =
                W1@t1 + Wc@feat; relu; W2; bias -> o_sl (bf16)."""
                X8 = X[:, nb - 1, :].bitcast(fp8e4)  # [128, 2G]
                X83 = X8.rearrange("p (j two) -> p two j", two=2)
                p_h1 = ps2.tile([128, SMAX], f32, tag="h1")
                nc.tensor.matmul(p_h1[:, :S], lhsT=W.w1t[:], rhs=rhs_t1,
                                 start=True, stop=False)
                nc.tensor.matmul(p_h1[:, :S], lhsT=W.wfdr[:],
                                 rhs=X83[:, :, bass.ds(c0, S)],
                                 start=False, stop=True,
                                 perf_mode=mybir.MatmulPerfMode.DoubleRow)
                h1 = sb.tile([128, SMAX], bf16, tag="h1s")
                nc.scalar.activation(h1[:, :S], p_h1[:, :S], AF.Relu,
                                     bias=W.b1p[:], scale=W.sbig[:])
                p_o = ps2.tile([64, SMAX], f32, tag="out")
                nc.tensor.matmul(p_o[:, :S], lhsT=W.w2t[:], rhs=h1[:, :S],
                                 start=True, stop=True)
                nc.vector.tensor_scalar_add(o_sl, p_o[0:C, :S], W.b2v[:, 0:1])

            def upair(sb, ps2, X, c0a, S, Sb, osb):
                """Two equal untouched slices of one gather, weight-batched.
                The two W2 outputs land in one PSUM tile at partition rows
                0:64 and 64:128 (tile_position col packing) so one DVE op
                applies the bias for both."""
                c0b = c0a + S
                X8 = X[:, 1, :].bitcast(fp8e4)
                X83 = X8.rearrange("p (j two) -> p two j", two=2)
                ph_a = ps2.tile([128, SMAX], f32, tag="h1a")
                nc.tensor.matmul(ph_a[:, :S], lhsT=W.w1t[:],
                                 rhs=X[:, 0, bass.ds(c0a, S)],
                                 start=True, stop=False)
                nc.tensor.matmul(ph_a[:, :S], lhsT=W.wfdr[:],
                                 rhs=X83[:, :, bass.ds(c0a, S)],
                                 start=False, stop=True,
                                 perf_mode=mybir.MatmulPerfMode.DoubleRow)
                h1a = sb.tile([128, SMAX], bf16, tag="h1sa")
                nc.scalar.activation(h1a[:, :S], ph_a[:, :S], AF.Relu,
                                     bias=W.b1p[:], scale=W.sbig[:])
                if Sb:
                    ph_b = ps2.tile([128, SMAX], f32, tag="h1b")
                    nc.tensor.matmul(ph_b[:, :S], lhsT=W.w1t[:],
                                     rhs=X[:, 0, bass.ds(c0b, S)],
                                     start=True, stop=False)
                    nc.tensor.matmul(ph_b[:, :S], lhsT=W.wfdr[:],
                                     rhs=X83[:, :, bass.ds(c0b, S)],
                                     start=False, stop=True,
                                     perf_mode=mybir.MatmulPerfMode.DoubleRow)
                    h1b = sb.tile([128, SMAX], bf16, tag="h1sb")
                    nc.scalar.activation(h1b[:, :S], ph_b[:, :S], AF.Relu,
                                         bias=W.b1p[:], scale=W.sbig[:])
                p_oa = ps2.tile([64, SMAX], f32, tag="out2a")
                nc.tensor.matmul(p_oa[:, :S], lhsT=W.w2t[:], rhs=h1a[:, :S],
                                 start=True, stop=True)
                nc.vector.tensor_scalar_add(
                    osb[:, :S], p_oa[0:C, :S], W.b2v[:, 0:1])
                if Sb:
                    p_ob = ps2.tile([64, SMAX], f32, tag="out2b")
                    nc.tensor.matmul(p_ob[:, :S], lhsT=W.w2t[:],
                                     rhs=h1b[:, :S], start=True, stop=True)
                    nc.vector.tensor_scalar_add(
                        osb[:, S:2 * S], p_ob[0:C, :S], W.b2v[:, 0:1])

            def slice_t(sb, ps, ps2, X, c0, S, o_sl):
                sl = bass.ds(c0, S)
                stT = X[:, 1, sl]
                p_r = ps.tile([128, SMAX], f32, tag="gr")
                nc.tensor.matmul(p_r[:, :S], lhsT=W.whhrt[:], rhs=stT,
                                 start=True, stop=True)
                p_z = ps.tile([128, SMAX], f32, tag="gz")
                nc.tensor.matmul(p_z[:, :S], lhsT=W.whhzt[:], rhs=stT,
                                 start=True, stop=True)
                p_n = ps.tile([128, SMAX], f32, tag="gn")
                nc.tensor.matmul(p_n[:, :S], lhsT=W.whhnt[:], rhs=stT,
                                 start=True, stop=True)
                r = sb.tile([128, SMAX], f32, tag="r")
                nc.scalar.activation(r[:, :S], p_r[:, :S], AF.Sigmoid,
                                     bias=W.c_r[:])
                z = sb.tile([128, SMAX], f32, tag="z")
                nc.scalar.activation(z[:, :S], p_z[:, :S], AF.Sigmoid,
                                     bias=W.c_z[:])
                hn = sb.tile([128, SMAX], f32, tag="hn")
                nc.scalar.activation(hn[:, :S], p_n[:, :S], AF.Identity,
                                     bias=W.bhn[:])
                rn = sb.tile([128, SMAX], f32, tag="rn")
                nc.vector.tensor_tensor(out=rn[:, :S], in0=r[:, :S],
                                        in1=hn[:, :S], op=OP.mult)
                n = sb.tile([128, SMAX], f32, tag="n")
                nc.scalar.activation(n[:, :S], rn[:, :S], AF.Tanh,
                                     bias=W.gin[:])
                d = sb.tile([128, SMAX], f32, tag="d")
                nc.vector.tensor_tensor(out=d[:, :S], in0=stT, in1=n[:, :S],
                                        op=OP.subtract)
                zd = sb.tile([128, SMAX], f32, tag="zd")
                nc.vector.tensor_tensor(out=zd[:, :S], in0=z[:, :S],
                                        in1=d[:, :S], op=OP.mult)
                ns = sb.tile([128, SMAX], f32, tag="ns")
                nc.vector.tensor_tensor(out=ns[:, :S], in0=n[:, :S],
                                        in1=zd[:, :S], op=OP.add)
                sstate = sb.tile([128, SMAX], bf16, tag="sstate")
                nc.vector.tensor_scalar_mul(sstate[:, :S], ns[:, :S],
                                            W.dect[:, 0:1])
                t1 = sb.tile([128, SMAX], bf16, tag="t1")
                nc.vector.tensor_tensor(out=t1[:, :S], in0=sstate[:, :S],
                                        in1=X[:, 0, sl], op=OP.add)
                tail(sb, ps2, X, 3, c0, S, t1[:, :S], o_sl)

            qctr = [0]

            def gather(gp, table_ap, idx_tile, goff, g, nb, tag, bufs):
                q = qctr[0] % NQ
                qctr[0] += 1
                X = gp.tile([128, nb, g], bf16, tag=f"{tag}{q}_{g}",
                            bufs=bufs)
                nc.gpsimd.dma_gather(
                    out_ap=X[:],
                    in_ap=table_ap[:],
                    idxs_ap=idx_tile[:, bass.ds(goff // 16, g // 16)],
                    num_idxs=g,
                    num_idxs_reg=g,
                    elem_size=nb * 128,
                    transpose=True,
                    queue_num=q,
                )
                return X

            def tstream(gp, sb, ps, ps2, table_ap, idx_tile, gsizes, col0):
                goff = 0
                for g in gsizes:
                    X = gather(gp, table_ap, idx_tile, goff, g, 3, "gt", 2)
                    osb = gp.tile([C, G_T], bf16, tag="osbt", bufs=2)
                    c0 = 0
                    for S in _sched(g, SMAX):
                        slice_t(sb, ps, ps2, X, c0, S,
                                osb[:, bass.ds(c0, S)])
                        c0 += S
                    nc.sync.dma_start(
                        out[:, bass.ds(col0 + goff, g)], osb[:, :g])
                    goff += g

            def ugather(gp, table_ap, idx_tile, goff, g):
                return gather(gp, table_ap, idx_tile, goff, g, 2, "gu",
                              4 if g == G_U else 2)

            def ustream(gp, sb, ps2, table_ap, idx_tile, col0, pre=()):
                goff = 0
                for gi, g in enumerate(u_gs):
                    if gi < len(pre):
                        X = pre[gi]
                    else:
                        X = ugather(gp, table_ap, idx_tile, goff, g)
                    if g > SMAX:
                        Sa = Sb = g // 2
                    else:
                        Sa, Sb = g, 0
                    osb = gp.tile([C, G_U], bf16, tag="osbu", bufs=6)
                    upair(sb, ps2, X, 0, Sa, Sb, osb)
                    nc.sync.dma_start(
                        out[:, bass.ds(col0 + goff, g)], osb[:, :g])
                    goff += g

            # PE warm-up burst: dummy matmuls against the already-loaded
            # uidx bytes (bitcast bf16) keep the PE HAM active through the
            # gather startup so real matmuls run un-throttled (2.4 GHz).
            with tc.tile_pool(name="wmps", bufs=1, space="PSUM") as wps:
                wrhs = W.uidx_a[:].bitcast(bf16)
                wcols = min(SMAX, u_pad // 16)
                pwarm = wps.tile([128, SMAX], f32, tag="warm")
                for _ in range(16):
                    nc.tensor.matmul(pwarm[:, :wcols], lhsT=W.w1t[:],
                                     rhs=wrhs[:, :wcols],
                                     start=True, stop=True)

            # Touched first: its serial GRU chain drains while untouched
            # gathers already issue from the shared gather pool. A few
            # untouched gathers are pre-issued so all 4 SWDGE queues have
            # work from the start.
            with tc.tile_pool(name="gp", bufs=4) as gp:
                pre_a = []
                with tc.tile_pool(name="sbt", bufs=2) as sb, \
                     tc.tile_pool(name="pst", bufs=1, space="PSUM") as ps, \
                     tc.tile_pool(name="pst2", bufs=2, space="PSUM") as ps2:
                    tstream(gp, sb, ps, ps2, tabt_a, W.tidx_a, t_gs, 0)
                    tstream(gp, sb, ps, ps2, tabt_b, W.tidx_b, t_gs, t_pad)
                with tc.tile_pool(name="sbu", bufs=3) as sb, \
                     tc.tile_pool(name="psu2", bufs=2, space="PSUM") as ps2:
                    ustream(gp, sb, ps2, tab_a, W.uidx_a, 2 * t_pad,
                            pre=pre_a)
                    ustream(gp, sb, ps2, tab_b, W.uidx_b, 2 * t_pad + u_pad)

    nc.compile()
    return nc


@functools.lru_cache(maxsize=4)
def _cached_program(u_pad, t_pad, tt_rows):
    return build_program(u_pad, t_pad, tt_rows)


def _round_up(x, m):
    return ((x + m - 1) // m) * m


def _pack_feat_u16(feats: np.ndarray) -> np.ndarray:
    """fp8(e3m4, x32) encode features and pack byte pairs into uint16
    (little-endian: feature 2p low byte, 2p+1 high byte -> unit p)."""
    f8 = (feats.astype(np.float32) * F8SCALE).astype(FP8).view(np.uint8)
    return f8[:, 0::2].astype(np.uint16) | (
        f8[:, 1::2].astype(np.uint16) << 8)


def _prepare(label_nodes, src, dst, t, msg, memory_buf, node_state, last_seen,
             node_features, decay, W_msg, b_msg, W_ih, W_hh, b_ih, b_hh,
             W_feat, b_feat, W1, b1, W2, b2, current_time):
    """Host-side routing/packing. Returns (in_maps, meta)."""
    label_nodes = np.asarray(label_nodes)

    # ---- host: event-level scalars (O(1) work) ----
    t0 = float(np.asarray(t)[0])
    T = float(current_time)
    rdecay = max(float(decay), 0.0)
    event_msg = msg[0].astype(np.float64) @ W_msg.T.astype(np.float64) + b_msg
    gi = event_msg @ W_ih.T.astype(np.float64) + b_ih  # [3H], includes b_ih
    gi = gi.astype(np.float32)
    dec_t = np.float32(np.exp(-rdecay * (T - t0)))

    # ---- host: routing (dedup to unique label nodes) ----
    touched_nodes = np.unique(np.concatenate([src, dst]))
    uniq_vals, inv = np.unique(label_nodes, return_inverse=True)
    is_t = np.isin(uniq_vals, touched_nodes)
    chunk_id = uniq_vals // CH            # 0..15
    local = (uniq_vals % CH).astype(np.int16)

    key = chunk_id.astype(np.int64) * 2 + is_t
    order = np.argsort(key, kind="stable")
    counts = np.bincount(key, minlength=NCHUNK * 2)
    u_counts = counts[0::2]
    t_counts = counts[1::2]
    u_pad = max(_round_up(int(u_counts.max()), 128), 128)
    t_pad = max(_round_up(int(t_counts.max()), 128), 128)

    starts = np.zeros(NCHUNK * 2 + 1, dtype=np.int64)
    np.cumsum(counts, out=starts[1:])
    group_uids = {}  # (chunk, touched) -> unique-label ids in device order
    for ci in range(NCHUNK):
        for tf in (0, 1):
            k = ci * 2 + tf
            group_uids[(ci, tf)] = order[starts[k]:starts[k + 1]]

    # ---- host: untouched packed table [N, 512B] ----
    # combined = mem + state * exp(-decay*(T - last_seen)) (valid when the
    # event did not touch the node; touched labels never read this table)
    dec_all = np.exp(-rdecay * (T - last_seen.astype(np.float32)))
    comb = (memory_buf.astype(np.float32)
            + node_state.astype(np.float32) * dec_all[:, None]).astype(BF16)
    tab = np.zeros((N, ROW_U), dtype=np.uint16)
    tab[:, 0:128] = comb.view(np.uint16)
    tab[:, 128:214] = _pack_feat_u16(node_features)
    tab = tab.view(BF16)

    # ---- host: touched compact tables (label-touched rows per chunk, in
    # device order -> gather indices are just arange) ----
    tt_rows = 16
    t_rows = {}
    for ci in range(NCHUNK):
        nodes = uniq_vals[group_uids[(ci, 1)]]
        t_rows[ci] = nodes
        tt_rows = max(tt_rows, _round_up(nodes.shape[0], 16))

    def build_tabt(nodes):
        tt = np.zeros((tt_rows, ROW_T), dtype=np.uint16)
        nr = nodes.shape[0]
        if nr:
            tt[:nr, 0:128] = memory_buf[nodes].astype(BF16).view(np.uint16)
            tt[:nr, 128:256] = node_state[nodes].astype(BF16).view(np.uint16)
            tt[:nr, 256:342] = _pack_feat_u16(node_features[nodes])
        return tt.view(BF16)

    # ---- host: weights / aux ----
    def bfc(x):
        return np.ascontiguousarray(x, dtype=BF16)

    def f32c(x):
        return np.ascontiguousarray(x, dtype=np.float32).reshape(-1, 1)

    WcT = (W1 @ W_feat).T  # [F, H] — W_feat folded through W1
    wfdr = np.zeros((128, 2, 128), dtype=FP8)
    wfdr.reshape(256, 128)[0:F] = (WcT * F8SCALE).astype(np.float32).astype(FP8)
    aux = {
        "w1t": bfc(W1.T * BIG),
        "wfdr": wfdr,
        "w2t": bfc(np.concatenate(
            [W2.T, np.zeros((128, 64 - C), np.float32)], axis=1)),
        "whhrt": bfc(W_hh[0:128].T),
        "whhzt": bfc(W_hh[128:256].T),
        "whhnt": bfc(W_hh[256:384].T),
        "b1p": f32c(b1 + W1 @ b_feat),
        "b2v": f32c(b2),
        "b2d": f32c(np.concatenate(
            [b2, np.zeros(14, np.float32), b2, np.zeros(14, np.float32)])),
        "c_r": f32c(gi[0:128] + b_hh[0:128]),
        "c_z": f32c(gi[128:256] + b_hh[128:256]),
        "gin": f32c(gi[256:384]),
        "bhn": f32c(b_hh[256:384]),
        "dect": np.full((128, 1), dec_t, dtype=np.float32),
        "sbig": np.full((128, 1), 1.0 / BIG, dtype=np.float32),
    }

    def uidx_input(ci):
        uids = group_uids[(ci, 0)]
        li = np.zeros(u_pad, dtype=np.int16)
        li[:uids.shape[0]] = local[uids]
        return _wrap_idxs(li)

    def tidx_input(ci):
        li = np.zeros(t_pad, dtype=np.int16)
        nr = group_uids[(ci, 1)].shape[0]
        li[:nr] = np.arange(nr, dtype=np.int16)
        return _wrap_idxs(li)

    in_maps = []
    for core in range(NCORES):
        ca, cb = 2 * core, 2 * core + 1
        im = dict(aux)
        im["tab_a"] = tab[ca * CH:(ca + 1) * CH]
        im["tab_b"] = tab[cb * CH:(cb + 1) * CH]
        im["tabt_a"] = build_tabt(t_rows[ca])
        im["tabt_b"] = build_tabt(t_rows[cb])
        im["uidx_a"] = uidx_input(ca)
        im["uidx_b"] = uidx_input(cb)
        im["tidx_a"] = tidx_input(ca)
        im["tidx_b"] = tidx_input(cb)
        in_maps.append(im)

    # column (within a core's output) of each unique label node
    totcol = 2 * (u_pad + t_pad)
    col_of_uniq = np.empty(uniq_vals.shape[0], dtype=np.int64)
    for ci in range(NCHUNK):
        core = ci // 2
        for tf in (0, 1):
            uids = group_uids[(ci, tf)]
            if tf == 1:
                c0 = 0 if (ci % 2) == 0 else t_pad
            else:
                c0 = 2 * t_pad if (ci % 2) == 0 else 2 * t_pad + u_pad
            col_of_uniq[uids] = core * totcol + c0 + np.arange(uids.shape[0])

    meta = {"u_pad": u_pad, "t_pad": t_pad, "tt_rows": tt_rows,
            "col_of_uniq": col_of_uniq, "inv": inv, "nb": label_nodes.shape[0]}
    return in_maps, meta


def _finish(core_outs, meta):
    """Map per-core [C, totcol] bf16 outputs back to label order (f32)."""
    combined = np.concatenate(core_outs, axis=1)  # [C, NCORES*totcol]
    sel = combined[:, meta["col_of_uniq"][meta["inv"]]].T
    return np.ascontiguousarray(sel, dtype=np.float32)


def kernel(**inputs):
    inputs = {k: np.asarray(v) for k, v in inputs.items()}
    in_maps, meta = _prepare(**inputs)
    nc = _cached_program(meta["u_pad"], meta["t_pad"], meta["tt_rows"])
    res = run_bass_kernel_spmd(nc, in_maps, core_ids=list(range(NCORES)))
    return _finish([r["out"] for r in res.results], meta)
